# revision 1
# baseline (speedup 1.0000x reference)
"""MultiHeadAttention Trainium2 kernel.

Problem (hardcoded): S=2048, B=2, D=1024, H=16, HD=64, fp32 I/O.
  q = query @ w_q.T + b_q   (same for k, v), heads split from D
  scores[i,j,b,h] = (q_i . k_j)/8, masked where mask[j]==0, softmax over j
  out[i,b,:] = concat_h( sum_j p_ij v_j )

Sharding: 8 cores = 2 batches x 4 head-groups (4 heads / 256 dims each).
Host-side prep: cast to bf16, transpose to [D, seq] layout, and compact the
key/value sequence to the unmasked positions only (masked j contribute
exactly 0 after softmax), padded to a multiple of 128.

Per-core program (Tile framework), engineered so the ACT engine (exp) is
saturated once the input stream lands and PE work hides beneath it:
  - Q,K projections k-outer (start as soon as the first k-tile lands),
    output qT/kT[o, i] with o (head dims) on partitions, bf16.
  - V projection flipped (x^T tiles stationary) giving V[j, o] with j on
    partitions; V_ext adds a per-head mask column (1 real / 0 padding) so
    the softmax denominator falls out of the PV matmul.
  - Scores transposed: S^T[j, i] = kT.T @ qT; the two heads of a pair run
    CONCURRENTLY on disjoint PE row halves (explicit tile_position (0,0) /
    (64,0)) - verified ~2x on HW - since c=64 only half-fills the array.
  - P^T = exp(S^T / 8) on ACT in [128, 1024] chunks, written into per-pair
    [128, 2*IBLK] bf16 tiles (halves the P^T buffer count).
  - PV: out^T[vd, i] += V_ext[j,:].T @ P^T[j, i-chunk], fp32 PSUM
    accumulation over j.  m=65 (64 v-dims + denominator) is intentionally
    NOT head-packed: any separate denominator matmul would re-stream P^T
    and exactly cancel the packing gain.
  - Output staged per head as [65, IBLK] bf16 and DMA'd in one shot per
    head per i-block (host folds the softmax division + transpose into
    unsharding).
  - Emission is software-pipelined: ahead of the first score block only
    K-proj j0-sliver + Q-proj first half run; everything else (V proj,
    remaining projections, each phase's PV groups) is a cost-annotated
    "filler" placed inside later phases' exp windows, popped per j-tile
    with a dynamic budget so the PE never starves the ACT engine.
    Filler piece sizes are tuned to the ~2.2us exp-pair slot budget (HW
    swept): PV chains in j-tile THIRDS (~0.64us; halves and quarters both
    measured slower), single-j-tile V waves, 256-wide mid-phase Q passes,
    448-wide K passes (finer K regressed - those slots are DMA-gated), a
    small piece leading each list, start_jt=0 everywhere, and one V wave
    moved into ph00's slack to balance the tightest phase.  Measured
    ~97% ACT occupancy; mean 125us/core (was 133 at the coarse-filler
    stage).
  - DMA notes (measured): each dma_start binds ~one of 16 DMA engines
    (~22.5 GB/s), issue costs ~0.65us on the issuing queue, and there is a
    ~6us framework preamble plus a ~10us fixed exit barrier; the per-k
    0.26MB transfer granularity balances engine parallelism against issue
    serialization (big merged DMAs and fine splits both measured slower).
"""

import math
import sys

sys.path.insert(0, "/opt/trn_rl_repo")

import numpy as np
import ml_dtypes

import concourse.tile as tile
from concourse import bacc, mybir
from concourse.bass_utils import run_bass_kernel_spmd

S, B, D, H, HD = 2048, 2, 1024, 16, 64
N_CORES = 8
GROUPS = 4          # head groups (cores per batch)
GH = H // GROUPS    # heads per core = 4
GD = GH * HD        # dims per core = 256
KT = D // 128       # contraction k-tiles = 8
IBLK = 1024         # i block (exp granularity / P^T tile width)
NIB = S // IBLK     # i blocks = 2
VW1 = HD + 1        # per-head vext width (64 v cols + denominator col)

BF16 = mybir.dt.bfloat16
F32 = mybir.dt.float32
F32R = mybir.dt.float32r
EXP = mybir.ActivationFunctionType.Exp

_CACHE = {}


def _chunks(total, step):
    out = []
    o = 0
    while o < total:
        n = min(step, total - o)
        out.append((o, n))
        o += n
    return out


def _pairs(seq):
    return [tuple(seq[i:i + 2]) for i in range(0, len(seq), 2)]


def _build(J, J_real, use_bias):
    """Build + compile the per-core Bass program (identical on all cores)."""
    NJT = J // 128
    nc = bacc.Bacc("TRN2", target_bir_lowering=False, debug=False,
                   enable_asserts=False)

    xq_d = nc.dram_tensor("xq", (D, S), BF16, kind="ExternalInput")
    xk_d = nc.dram_tensor("xk", (D, J), BF16, kind="ExternalInput")
    xv_d = nc.dram_tensor("xv", (D, J), BF16, kind="ExternalInput")
    wq_d = nc.dram_tensor("wq", (128, KT * GD), BF16, kind="ExternalInput")
    wk_d = nc.dram_tensor("wk", (128, KT * GD), BF16, kind="ExternalInput")
    wv_d = nc.dram_tensor("wv", (128, KT * GD), BF16, kind="ExternalInput")
    mpad_d = nc.dram_tensor("mpad", (128, NJT), BF16, kind="ExternalInput")
    if use_bias:
        bq_d = nc.dram_tensor("bq", (GD, 1), F32, kind="ExternalInput")
        bk_d = nc.dram_tensor("bk", (GD, 1), F32, kind="ExternalInput")
        bv_d = nc.dram_tensor("bv", (1, GD), BF16, kind="ExternalInput")
    out_d = nc.dram_tensor("out", (GH * VW1, S), BF16, kind="ExternalOutput")

    # SBUF budget for the P^T pool, in per-partition bytes (SBUF tiles
    # reserve their free-dim byte span on every partition).
    fixed_pp = (KT * S * 2                 # xq tiles
                + 2 * KT * J * 2           # xk, xv tiles
                + 3 * KT * GD * 2          # weights
                + 2 * S * 2 + 2 * J * 2    # qT/kT pool
                + NJT * (GH * VW1 + 8) * 2   # vext
                + 3 * IBLK * 2             # out staging (bf16)
                + 5 * 1024)                # consts, mpad, small, slack
    budget_pp = 188 * 1024 - fixed_pp
    # pt holds (h0|h1) pair tiles [128, 2*IBLK]; pipelining needs two score
    # phases' worth alive (2*NJT) plus a little slack
    pt_bufs = max(NJT // 2 + 2, min(2 * NJT + 3, budget_pp // (2 * IBLK * 2)))
    pipelined = pt_bufs >= 2 * NJT + 1

    scale = 1.0 / math.sqrt(HD)  # 0.125, folded into the exp

    with tile.TileContext(nc) as tc:
        with (
            tc.tile_pool(name="xq", bufs=KT) as xq_p,
            tc.tile_pool(name="xk", bufs=KT) as xk_p,
            tc.tile_pool(name="xv", bufs=KT) as xv_p,
            tc.tile_pool(name="w", bufs=3) as w_p,
            tc.tile_pool(name="qk", bufs=2) as qk_p,
            tc.tile_pool(name="vext", bufs=NJT) as vext_p,
            tc.tile_pool(name="pt", bufs=pt_bufs) as pt_p,
            tc.tile_pool(name="small", bufs=10) as small_p,
            tc.tile_pool(name="ost", bufs=3) as ost_p,
            tc.tile_pool(name="sps", bufs=3, space="PSUM") as sps_p,
            tc.tile_pool(name="pps", bufs=2, space="PSUM") as pps_p,
        ):
            # ---- input DMAs ----
            # Order tuned for the earliest possible first exp: weights, then
            # the j 0:128 slivers of xk (k-outer K proj of j-tile 0 can
            # finish ~3.7us in), then xq first halves (i 0:1024), then the
            # rest of xk.  ACT's first [128,1024] exp fires once kT(jt0) and
            # qT(i 0:1024) exist.
            def load_w(w_d, eng):
                w_sb = w_p.tile([128, KT * GD], BF16, tag="w", name="w_sb")
                eng.dma_start(w_sb[:], w_d.ap())
                return w_sb

            wk_sb = load_w(wk_d, nc.sync)
            wq_sb = load_w(wq_d, nc.scalar)
            mpad_sb = small_p.tile([128, NJT], BF16, tag="mpad")
            nc.sync.dma_start(mpad_sb[:], mpad_d.ap())
            # Strict HBM priority via issue windows: until ~13us ONLY the
            # first-exp critical set is in flight (wk, wq, xk j0-sliver, xq
            # first halves); the rest is issued after it on both queues.
            j0 = min(256, J)
            xk_t = []
            xq_t = []
            for k in range(KT):
                t = xk_p.tile([128, J], BF16, tag="xk", name=f"xk{k}")
                nc.sync.dma_start(t[:, 0:j0],
                                  xk_d.ap()[k * 128:(k + 1) * 128, 0:j0])
                xk_t.append(t)
            for k in range(KT):
                t = xq_p.tile([128, S], BF16, tag="xq", name=f"xq{k}")
                nc.scalar.dma_start(t[:, 0:1024],
                                    xq_d.ap()[k * 128:(k + 1) * 128, 0:1024])
                xq_t.append(t)
            for k in range(KT):
                if J > j0:
                    nc.sync.dma_start(xk_t[k][:, j0:J],
                                      xk_d.ap()[k * 128:(k + 1) * 128, j0:J])
            wv_sb = load_w(wv_d, nc.sync)
            xv_t = []
            for k in range(KT):
                t = xv_p.tile([128, J], BF16, tag="xv", name=f"xv{k}")
                nc.sync.dma_start(t[:], xv_d.ap()[k * 128:(k + 1) * 128, :])
                xv_t.append(t)
            for k in range(KT):
                nc.scalar.dma_start(xq_t[k][:, 1024:S],
                                    xq_d.ap()[k * 128:(k + 1) * 128, 1024:S])
            if use_bias:
                bq_c = small_p.tile([128, 2], F32, tag="biasq")
                nc.sync.dma_start(
                    bq_c[:].rearrange("p (o x) -> p o x", o=2),
                    bq_d.ap().rearrange("(o p) x -> p o x", p=128))
                bk_c = small_p.tile([128, 2], F32, tag="biask")
                nc.sync.dma_start(
                    bk_c[:].rearrange("p (o x) -> p o x", o=2),
                    bk_d.ap().rearrange("(o p) x -> p o x", p=128))
                bv_row = small_p.tile([1, GD], BF16, tag="bvrow")
                nc.sync.dma_start(bv_row[:], bv_d.ap())
                ones_row = small_p.tile([1, 128], BF16, tag="ones")
                nc.vector.memset(ones_row[:], 1.0)

            # prime the ACT exp table during the initial DMA window
            warm = small_p.tile([1, 8], F32, tag="warm")
            nc.vector.memset(warm[:], 0.0)
            warm2 = small_p.tile([1, 8], F32, tag="warm2")
            nc.scalar.activation(warm2[:], warm[:], EXP, scale=1.0)

            # ---- projections ----
            qT = {}   # per otile: [128, S] bf16  (o on partitions)
            kTt = {}  # per otile: [128, J] bf16

            def proj_pass(x_tiles, w_sb, dst, bias_col, ot, chunk_group):
                """One k-outer accumulation pass over <=2 width-chunks."""
                ps = [pps_p.tile([128, 512], F32, tag="pps",
                                 name=f"pps{ot}{o0}") for (o0, _) in chunk_group]
                for k in range(KT):
                    lw = w_sb[:, k * GD + ot * 128:k * GD + (ot + 1) * 128]
                    for ci, (o0, n) in enumerate(chunk_group):
                        nc.tensor.matmul(ps[ci][:, 0:n], lhsT=lw,
                                         rhs=x_tiles[k][:, o0:o0 + n],
                                         start=(k == 0), stop=(k == KT - 1))
                for ci, (o0, n) in enumerate(chunk_group):
                    if use_bias:
                        nc.vector.tensor_scalar(
                            dst[:, o0:o0 + n], ps[ci][:, 0:n],
                            bias_col[:, ot:ot + 1], None,
                            mybir.AluOpType.add)
                    else:
                        nc.vector.tensor_copy(dst[:, o0:o0 + n], ps[ci][:, 0:n])

            def proj_passes(x_tiles, w_sb, dst_map, bias_col, width, ot,
                            chunk_list=None):
                """Returns (cost_ns, closure) filler pieces, one per chunk
                pass."""
                dst = qk_p.tile([128, width], BF16,
                                tag="qt" if width == S else "kt",
                                name=f"qk{ot}")
                dst_map[ot] = dst
                if chunk_list is None:
                    chunk_list = _chunks(width, 512)
                return [
                    (int(KT * c[1] * 0.42),
                     lambda cg=cg: proj_pass(x_tiles, w_sb, dst, bias_col,
                                             ot, cg))
                    for c, cg in [(c, [c]) for c in chunk_list]
                ]

            vext = [None] * NJT

            def v_wave(jts):
                """V projection (flipped orientation) for a couple of j-tiles
                + V_ext assembly."""
                ps = [pps_p.tile([128, GD], F32, tag="pps", name=f"ppsv{jt}")
                      for jt in jts]
                for k in range(KT):
                    for vi, jt in enumerate(jts):
                        nc.tensor.matmul(
                            ps[vi][:, :],
                            lhsT=xv_t[k][:, jt * 128:(jt + 1) * 128],
                            rhs=wv_sb[:, k * GD:(k + 1) * GD],
                            start=(k == 0),
                            stop=(k == KT - 1) and not use_bias)
                for vi, jt in enumerate(jts):
                    if use_bias:
                        nc.tensor.matmul(ps[vi][:, :], lhsT=ones_row[:, :],
                                         rhs=bv_row[:, :], start=False,
                                         stop=True)
                    ve = vext_p.tile([128, GH * VW1], BF16, tag="vext",
                                     name=f"vext{jt}")
                    for h in range(GH):
                        nc.vector.tensor_copy(
                            ve[:, h * VW1:h * VW1 + HD],
                            ps[vi][:, h * HD:(h + 1) * HD])
                        nc.vector.tensor_copy(
                            ve[:, h * VW1 + HD:h * VW1 + HD + 1],
                            mpad_sb[:, jt:jt + 1])
                    vext[jt] = ve

            def emit_qkt(ib, hp, fillers, start_jt=3, split_jts=(),
                         mid_fillers=None):
                """Scores + exp for head pair hp of i-block ib. The A/B head
                matmuls go to disjoint PE row halves (explicit tile_position)
                so the pair streams concurrently at ~2x. Emits one filler
                closure after each j-tile (from start_jt on) to keep PE fed
                while ACT chews on the exps."""
                i0 = ib * IBLK
                pt = {}
                for jt in range(NJT):
                    psA = sps_p.tile([128, IBLK], F32, tag="sps",
                                     name=f"sA{ib}{hp}{jt}")
                    psB = sps_p.tile([128, IBLK], F32, tag="sps",
                                     name=f"sB{ib}{hp}{jt}")
                    ptp = pt_p.tile([128, 2 * IBLK], BF16, tag="pt",
                                    name=f"pt{ib}{hp}{jt}")
                    if jt in split_jts:
                        # chunk-paced: exp each 512-wide half as soon as its
                        # scores land (first-exp latency, front only)
                        for (o, n) in _chunks(IBLK, 512):
                            nc.tensor.matmul(
                                psA[:, o:o + n],
                                lhsT=kTt[hp][0:64, jt * 128:(jt + 1) * 128],
                                rhs=qT[hp][0:64, i0 + o:i0 + o + n],
                                start=True, stop=True, tile_position=(0, 0))
                            nc.tensor.matmul(
                                psB[:, o:o + n],
                                lhsT=kTt[hp][64:128, jt * 128:(jt + 1) * 128],
                                rhs=qT[hp][64:128, i0 + o:i0 + o + n],
                                start=True, stop=True, tile_position=(64, 0))
                            nc.scalar.activation(ptp[:, o:o + n],
                                                 psA[:, o:o + n], EXP,
                                                 scale=scale)
                            nc.scalar.activation(
                                ptp[:, IBLK + o:IBLK + o + n],
                                psB[:, o:o + n], EXP, scale=scale)
                            while mid_fillers:
                                mid_fillers.pop(0)[1]()
                        pt[(hp * 2, jt)] = (ptp, 0)
                        pt[(hp * 2 + 1, jt)] = (ptp, IBLK)
                        continue
                    for (o, n) in _chunks(IBLK, 512):
                        nc.tensor.matmul(
                            psA[:, o:o + n],
                            lhsT=kTt[hp][0:64, jt * 128:(jt + 1) * 128],
                            rhs=qT[hp][0:64, i0 + o:i0 + o + n],
                            start=True, stop=True, tile_position=(0, 0))
                        nc.tensor.matmul(
                            psB[:, o:o + n],
                            lhsT=kTt[hp][64:128, jt * 128:(jt + 1) * 128],
                            rhs=qT[hp][64:128, i0 + o:i0 + o + n],
                            start=True, stop=True, tile_position=(64, 0))
                    nc.scalar.activation(ptp[:, 0:IBLK], psA[:], EXP,
                                         scale=scale)
                    nc.scalar.activation(ptp[:, IBLK:2 * IBLK], psB[:], EXP,
                                         scale=scale)
                    pt[(hp * 2, jt)] = (ptp, 0)
                    pt[(hp * 2 + 1, jt)] = (ptp, IBLK)
                    if jt >= start_jt and fillers:
                        slots_left = NJT - jt
                        budget = sum(c for c, _ in fillers) // slots_left
                        spent = 0
                        while fillers and spent < budget:
                            c, fn = fillers.pop(0)
                            fn()
                            spent += c
                while fillers:
                    fillers.pop(0)[1]()
                return pt

            ost_cur = {}
            pv_cur = {}

            def pv_group(ib, hp, hl, icl, pt, jt_lo, jt_hi, first, last):
                """Part of one PV accumulation group (j-tiles jt_lo:jt_hi);
                on the last part the numerators + denominator row are staged
                per head and DMA'd out unnormalized (host folds the division
                into unsharding)."""
                h = hp * 2 + hl
                if first:
                    pv_cur[(ib, h, icl)] = pps_p.tile(
                        [VW1, 512], F32, tag="pps", name=f"pv{ib}{h}{icl}")
                pv = pv_cur[(ib, h, icl)]
                for jt in range(jt_lo, jt_hi):
                    tp, off = pt[(h, jt)]
                    nc.tensor.matmul(
                        pv[:, :],
                        lhsT=vext[jt][:, h * VW1:(h + 1) * VW1],
                        rhs=tp[:, off + icl * 512:off + (icl + 1) * 512],
                        start=(jt == 0), stop=(jt == NJT - 1))
                if not last:
                    return
                if (ib, h) not in ost_cur:
                    ost_cur[(ib, h)] = ost_p.tile([VW1, IBLK], BF16,
                                                  tag="ost", name=f"o{ib}{h}")
                osb = ost_cur[(ib, h)]
                nc.vector.tensor_copy(osb[:, icl * 512:(icl + 1) * 512],
                                      pv[:, :])
                if icl == IBLK // 512 - 1:
                    nc.gpsimd.dma_start(
                        out_d.ap()[h * VW1:(h + 1) * VW1,
                                   ib * IBLK:(ib + 1) * IBLK],
                        osb[:])

            # ---- emission schedule ----
            bqc = bq_c if use_bias else None
            bkc = bk_c if use_bias else None
            # K proj in j-column passes: a narrow jt0-1 pass first so the
            # first two score j-tiles start as early as possible, then
            # ~448-wide passes that ride behind the xk-rest DMA.
            kchunks = [(0, j0)]
            if J > j0:
                kchunks += [(j0 + o, n) for (o, n) in _chunks(J - j0, 448)]
            k0_passes = proj_passes(xk_t, wk_sb, kTt, bkc, J, 0, kchunks)
            k0_passes[0][1]()   # kT(hp0) j-tiles 0..2
            q0_passes = proj_passes(xq_t, wq_sb, qT, bqc, S, 0,
                                    _chunks(S, 256))
            for _, p in q0_passes[0:4]:
                p()          # i 0:1024 -> first score block can start

            v_fillers = [(int(KT * GD * 0.42),
                          lambda js=js: v_wave(js))
                         for js in [(jt,) for jt in range(NJT)]]
            q1_fillers = proj_passes(xq_t, wq_sb, qT, bqc, S, 1,
                                     _chunks(S, 256))
            k1_fillers = proj_passes(xk_t, wk_sb, kTt, bkc, J, 1, kchunks)

            def pv_fillers(ib, hp, pt):
                out = []
                cuts = [0, NJT // 3, 2 * NJT // 3, NJT]
                for hl in range(2):
                    for icl in range(IBLK // 512):
                        for ci in range(3):
                            lo, hi = cuts[ci], cuts[ci + 1]
                            out.append((int((hi - lo) * 512 * 0.42),
                                        lambda hl=hl, icl=icl, lo=lo, hi=hi,
                                        ci=ci:
                                        pv_group(ib, hp, hl, icl, pt,
                                                 lo, hi, ci == 0, ci == 2)))
                return out

            def pv_tail(ib, hp, pt):
                """Final-phase PV: the two icl-groups of each head accumulate
                in lockstep per j-tile so only the last j-tile's matmuls
                trail the final exps."""
                # all four chains advance per j-tile so only the last
                # exp's head trails; h1's chains borrow the score-psum pool
                # (its buffers idle as the final exps drain them)
                pvs = {}
                for icl in range(IBLK // 512):
                    pvs[(0, icl)] = pps_p.tile([VW1, 512], F32, tag="pps",
                                               name=f"pvt0{icl}")
                for icl in range(IBLK // 512):
                    pvs[(1, icl)] = sps_p.tile([VW1, 512], F32, tag="sps",
                                               name=f"pvt1{icl}")
                for jt in range(NJT):
                    for hl in range(2):
                        h = hp * 2 + hl
                        for icl in range(IBLK // 512):
                            tp, off = pt[(h, jt)]
                            nc.tensor.matmul(
                                pvs[(hl, icl)][:, :],
                                lhsT=vext[jt][:, h * VW1:(h + 1) * VW1],
                                rhs=tp[:, off + icl * 512:
                                       off + (icl + 1) * 512],
                                start=(jt == 0), stop=(jt == NJT - 1))
                for hl in range(2):
                    h = hp * 2 + hl
                    osb = ost_p.tile([VW1, IBLK], BF16, tag="ost",
                                     name=f"ot{h}")
                    for icl in range(IBLK // 512):
                        nc.vector.tensor_copy(
                            osb[:, icl * 512:(icl + 1) * 512],
                            pvs[(hl, icl)][:, :])
                    nc.gpsimd.dma_start(
                        out_d.ap()[h * VW1:(h + 1) * VW1,
                                   ib * IBLK:(ib + 1) * IBLK],
                        osb[:])

            if pipelined:
                # Front = K0 jt0-1 + Q0 first half; everything else is
                # filler inside a later phase's exp window, ordered by both
                # DMA landing time and first use.  IMPORTANT: a kT pass must
                # be EMITTED before any scores j-tile that reads it (the PE
                # queue is in-order), hence k0-rest leads the ph00 fillers
                # with start_jt=0: k0[1] (covering jt2+) pops right after
                # the jt0 scores.
                pt00 = emit_qkt(0, 0,
                                k1_fillers[:1] + k0_passes[1:]
                                + k1_fillers[1:] + q0_passes[4:]
                                + q1_fillers[:4] + v_fillers[:1],
                                start_jt=0)
                g00 = pv_fillers(0, 0, pt00)
                pt10 = emit_qkt(1, 0, v_fillers[1:] + g00, start_jt=0)
                g10 = pv_fillers(1, 0, pt10)
                pt01 = emit_qkt(0, 1, g10 + q1_fillers[4:], start_jt=0)
                g01 = pv_fillers(0, 1, pt01)
                pt11 = emit_qkt(1, 1, g01, start_jt=0)
                pv_tail(1, 1, pt11)
            else:
                for _, p in k0_passes[1:] + q0_passes[4:]:
                    p()
                pt00 = emit_qkt(0, 0, [])
                for _, f in v_fillers + q1_fillers + k1_fillers:
                    f()
                for _, f in pv_fillers(0, 0, pt00):
                    f()
                pt10 = emit_qkt(1, 0, [])
                for _, f in pv_fillers(1, 0, pt10):
                    f()
                pt01 = emit_qkt(0, 1, [])
                for _, f in pv_fillers(0, 1, pt01):
                    f()
                pt11 = emit_qkt(1, 1, [])
                pv_tail(1, 1, pt11)

    nc.compile()
    return nc


def _prep_and_run(inputs, trace=False):
    query = np.asarray(inputs["query"], dtype=np.float32)
    key = np.asarray(inputs["key"], dtype=np.float32)
    value = np.asarray(inputs["value"], dtype=np.float32)
    mask = np.asarray(inputs["mask"]).reshape(S)
    w_q = np.asarray(inputs["w_q"], dtype=np.float32)
    b_q = np.asarray(inputs["b_q"], dtype=np.float32)
    w_k = np.asarray(inputs["w_k"], dtype=np.float32)
    b_k = np.asarray(inputs["b_k"], dtype=np.float32)
    w_v = np.asarray(inputs["w_v"], dtype=np.float32)
    b_v = np.asarray(inputs["b_v"], dtype=np.float32)

    use_bias = bool(np.any(b_q) or np.any(b_k) or np.any(b_v))

    # compact key/value over masked-out positions
    idx = np.nonzero(mask != 0)[0]
    J_real = int(len(idx))
    assert J_real > 0, "all positions masked: softmax undefined"
    J = max(512, ((J_real + 127) // 128) * 128)
    key_c = np.zeros((J, B, D), np.float32)
    key_c[:J_real] = key[idx]
    value_c = np.zeros((J, B, D), np.float32)
    value_c[:J_real] = value[idx]

    bf = ml_dtypes.bfloat16

    def wrearr(w):
        # [GD, D] -> w.T [D, GD] -> [128, KT*GD] (k-tiles side by side)
        KT_, GD_ = 8, w.shape[0]
        return np.ascontiguousarray(
            w.T.reshape(KT_, 128, GD_).transpose(1, 0, 2).reshape(
                128, KT_ * GD_)).astype(bf)

    NJT = J // 128
    mflat = np.zeros(J, np.float32)
    mflat[:J_real] = 1  # mpad[p, t] = 1 iff t*128+p < J_real
    mpad = np.ascontiguousarray(mflat.reshape(NJT, 128).T).astype(bf)
    in_maps = []
    for core in range(N_CORES):
        b = core // GROUPS
        g = core % GROUPS
        hs = slice(g * GD, (g + 1) * GD)
        m = {
            "xq": np.ascontiguousarray(query[:, b, :].T).astype(bf),
            "xk": np.ascontiguousarray(key_c[:, b, :].T).astype(bf),
            "xv": np.ascontiguousarray(value_c[:, b, :].T).astype(bf),
            "wq": wrearr(w_q[hs, :]),
            "wk": wrearr(w_k[hs, :]),
            "wv": wrearr(w_v[hs, :]),
            "mpad": mpad,
        }
        if use_bias:
            m["bq"] = np.ascontiguousarray(b_q[hs]).reshape(GD, 1)
            m["bk"] = np.ascontiguousarray(b_k[hs]).reshape(GD, 1)
            m["bv"] = np.ascontiguousarray(b_v[hs]).reshape(1, GD).astype(bf)
        in_maps.append(m)

    ck = (J, J_real, use_bias)
    if ck not in _CACHE:
        _CACHE[ck] = _build(J, J_real, use_bias)
    nc = _CACHE[ck]

    kwargs = {}
    if trace:
        kwargs = dict(trace=True, trace_cores=list(range(N_CORES)))
    res = run_bass_kernel_spmd(nc, in_maps, core_ids=list(range(N_CORES)),
                               **kwargs)

    out = np.empty((S, B, D), np.float32)
    for core in range(N_CORES):
        b = core // GROUPS
        g = core % GROUPS
        r = np.asarray(res.results[core]["out"]).astype(np.float32)
        r = r.reshape(GH, VW1, S)
        out[:, b, g * GD:(g + 1) * GD] = (
            (r[:, :HD, :] / r[:, HD:HD + 1, :])     # softmax denominator
            .reshape(GD, S).T)
    return out, res


def kernel(**inputs):
    out, _ = _prep_and_run(inputs, trace=False)
    return out


def run_traced(**inputs):
    _, res = _prep_and_run(inputs, trace=True)
    return res



# revision 14
# speedup vs baseline: 1.0684x; 1.0684x over previous
"""MultiHeadAttention Trainium2 kernel.

Problem (hardcoded): S=2048, B=2, D=1024, H=16, HD=64, fp32 I/O.
  q = query @ w_q.T + b_q   (same for k, v), heads split from D
  scores[i,j,b,h] = (q_i . k_j)/8, masked where mask[j]==0, softmax over j
  out[i,b,:] = concat_h( sum_j p_ij v_j )

Sharding: 8 cores = 2 batches x 4 head-groups (4 heads / 256 dims each).
Host-side prep: cast to bf16, transpose to [D, seq] layout, and compact the
key/value sequence to the unmasked positions only (masked j contribute
exactly 0 after softmax), padded to a multiple of 128.

Per-core program (Tile framework).  Trace-measured facts this schedule is
built on (J=1152): PE (TensorMatrix) is the saturated engine (~96% busy
13->130us in the old schedule); ACT exp busy is only ~76us; so exec time ~=
PE_start + PE_busy + tail.  DMA: each dma_start binds ~one of 16 engines
(~22.5 GB/s each), issue costs ~0.6us on the issuing sequencer queue; the
PE clock ramps (1.2 GHz until ~3us of continuous work, then 2.4 GHz).

Schedule:
  - Inputs are split into 64-192KB chunk DMAs issued in priority waves
    round-robin across the sync/vector/tensor/gpsimd queues (scalar stays
    clean for exps; vector/tensor only take early waves so their compute
    is not blocked behind 0.6us DMA issues).  Weight chunks are k-aligned
    so the k-outer projection passes chase the DMA stream.
  - Pre-phase PE: K proj j-tiles 0-1, K proj j-tiles 2-4 (chasing the
    xk-rest DMA), Q proj i 0:512, 512:1024.  First exps are chunk-paced
    (split_jts) so ACT starts ~13us in.
  - Scores transposed: S^T[j, i] = kT.T @ qT; the two heads of a pair run
    CONCURRENTLY on disjoint PE row halves (tile_position (0,0)/(64,0)).
  - P^T = exp(S^T / 8) on ACT in [128, 1024] chunks into per-pair
    [128, 2*IBLK] bf16 tiles.
  - PV: out^T[vd, i] += V_ext[j,:].T @ P^T[j, i-chunk], fp32 PSUM, m=65
    (64 v-dims + mask column so the softmax denominator falls out).
  - Fillers (cost-annotated closures popped per j-tile with a dynamic
    budget) keep the PE dense through every exp window; PV runs as
    j-tile-thirds pieces one phase behind its exps.
  - Outputs are staged per head as [65, IBLK] bf16 and DMA'd in 512-col
    halves as soon as each half's PSUM->SBUF copy lands, on alternating
    queues (kills the old single-engine 130KB tail DMA).
  - Host folds the softmax division + transpose into unsharding.
"""

import math
import sys

sys.path.insert(0, "/opt/trn_rl_repo")

import numpy as np
import ml_dtypes

import concourse.tile as tile
from concourse import bacc, mybir
from concourse.bass_utils import run_bass_kernel_spmd

S, B, D, H, HD = 2048, 2, 1024, 16, 64
N_CORES = 8
GROUPS = 4          # head groups (cores per batch)
GH = H // GROUPS    # heads per core = 4
GD = GH * HD        # dims per core = 256
KT = D // 128       # contraction k-tiles = 8
IBLK = 1024         # i block (exp granularity / P^T tile width)
NIB = S // IBLK     # i blocks = 2
VW1 = HD + 1        # per-head vext width (64 v cols + denominator col)

BF16 = mybir.dt.bfloat16
F32 = mybir.dt.float32
EXP = mybir.ActivationFunctionType.Exp

_CACHE = {}


def _chunks(total, step):
    out = []
    o = 0
    while o < total:
        n = min(step, total - o)
        out.append((o, n))
        o += n
    return out


def _blocks(total, widths):
    """Split [0, total) into blocks of the given widths (remainder gets one
    extra block).  Returns [(lo, hi, col_offset)] where col_offset is the
    block's start column in the k-interleaved packed layout."""
    out = []
    lo = 0
    off = 0
    for w in widths:
        hi = min(total, lo + w)
        if hi > lo:
            out.append((lo, hi, off))
            off += KT * (hi - lo)
        lo = hi
        if lo >= total:
            break
    if lo < total:
        out.append((lo, total, off))
    return out


def _build(J, J_real, use_bias):
    """Build + compile the per-core Bass program (identical on all cores)."""
    NJT = J // 128
    nc = bacc.Bacc("TRN2", target_bir_lowering=False, debug=False,
                   enable_asserts=False)

    xq_d = nc.dram_tensor("xq", (128, KT * S), BF16, kind="ExternalInput")
    xk_d = nc.dram_tensor("xk", (128, KT * J), BF16, kind="ExternalInput")
    xv_d = nc.dram_tensor("xv", (128, KT * J), BF16, kind="ExternalInput")
    wq_d = nc.dram_tensor("wq", (128, KT * GD), BF16, kind="ExternalInput")
    wk_d = nc.dram_tensor("wk", (128, KT * GD), BF16, kind="ExternalInput")
    wv_d = nc.dram_tensor("wv", (128, KT * GD), BF16, kind="ExternalInput")
    mpad_d = nc.dram_tensor("mpad", (128, NJT), BF16, kind="ExternalInput")
    if use_bias:
        bq_d = nc.dram_tensor("bq", (GD, 1), F32, kind="ExternalInput")
        bk_d = nc.dram_tensor("bk", (GD, 1), F32, kind="ExternalInput")
        bv_d = nc.dram_tensor("bv", (1, GD), BF16, kind="ExternalInput")
    out_d = nc.dram_tensor("out", (GH * VW1, S), BF16, kind="ExternalOutput")

    # SBUF budget for the P^T pool, in per-partition bytes (SBUF tiles
    # reserve their free-dim byte span on every partition).
    fixed_pp = (KT * S * 2                 # xq tiles
                + 2 * KT * J * 2           # xk, xv tiles
                + 3 * KT * GD * 2          # weights
                + 2 * S * 2 + 2 * J * 2    # qT/kT pool
                + NJT * (GH * VW1 + 8) * 2   # vext
                + 3 * IBLK * 2             # out staging (bf16)
                + 5 * 1024)                # consts, mpad, small, slack
    budget_pp = 188 * 1024 - fixed_pp
    # pt holds (h0|h1) pair tiles [128, 2*IBLK]; pipelining needs two score
    # phases' worth alive (2*NJT) plus a little slack
    pt_bufs = max(NJT // 2 + 2, min(2 * NJT + 3, budget_pp // (2 * IBLK * 2)))
    pipelined = pt_bufs >= 2 * NJT + 1

    scale = 1.0 / math.sqrt(HD)  # 0.125, folded into the exp

    with tile.TileContext(nc) as tc:
        with (
            tc.tile_pool(name="xq", bufs=1) as xq_p,
            tc.tile_pool(name="xk", bufs=1) as xk_p,
            tc.tile_pool(name="xv", bufs=1) as xv_p,
            tc.tile_pool(name="w", bufs=3) as w_p,
            tc.tile_pool(name="qk", bufs=2) as qk_p,
            tc.tile_pool(name="vext", bufs=NJT) as vext_p,
            tc.tile_pool(name="pt", bufs=pt_bufs) as pt_p,
            tc.tile_pool(name="small", bufs=10) as small_p,
            tc.tile_pool(name="ost", bufs=3) as ost_p,
            tc.tile_pool(name="sps", bufs=3, space="PSUM") as sps_p,
            tc.tile_pool(name="pps", bufs=2, space="PSUM") as pps_p,
        ):
            # ---- tiles ----
            # xq/xk/xv live in ONE SBUF tile each, in a host-packed BLOCKED
            # layout: block = a contiguous compute-ordered slab holding the
            # same column range of ALL KT k-tiles side by side.  One block =
            # one dma_start (a dma_start costs ~0.63us of sequencer issue
            # time regardless of size and internally fans out across all 16
            # SDMA engines, so few+large transfers win; HWDGE rings on the
            # sync/scalar queues drain FIFO, so issue order = landing order).
            wk_sb = w_p.tile([128, KT * GD], BF16, tag="w", name="wk_sb")
            wq_sb = w_p.tile([128, KT * GD], BF16, tag="w", name="wq_sb")
            wv_sb = w_p.tile([128, KT * GD], BF16, tag="w", name="wv_sb")
            mpad_sb = small_p.tile([128, NJT], BF16, tag="mpad")
            xq_all = xq_p.tile([128, KT * S], BF16, tag="xq", name="xq_all")
            xk_all = xk_p.tile([128, KT * J], BF16, tag="xk", name="xk_all")
            xv_all = xv_p.tile([128, KT * J], BF16, tag="xv", name="xv_all")

            QBLK = _blocks(S, (512, 512, 512, 512))
            KBLK = _blocks(J, (256, 512, 384))
            VBLK = _blocks(J, (640, 512))

            def _mkcol(blks):
                def col(k, lo, n):
                    for (b_lo, b_hi, off) in blks:
                        if b_lo <= lo and lo + n <= b_hi:
                            return off + k * (b_hi - b_lo) + (lo - b_lo)
                    raise AssertionError(f"span {lo}+{n} crosses blocks")
                return col

            qcol, kcol, vcol = _mkcol(QBLK), _mkcol(KBLK), _mkcol(VBLK)

            def xq_s(k, lo, n):
                c = qcol(k, lo, n)
                return xq_all[:, c:c + n]

            def xk_s(k, lo, n):
                c = kcol(k, lo, n)
                return xk_all[:, c:c + n]

            def xv_s(k, lo, n):
                c = vcol(k, lo, n)
                return xv_all[:, c:c + n]

            QS, QA, QG = nc.sync, nc.scalar, nc.gpsimd

            # prime the ACT exp table during the initial DMA window
            warm = small_p.tile([1, 8], F32, tag="warm")
            nc.vector.memset(warm[:], 0.0)
            warm2 = small_p.tile([1, 8], F32, tag="warm2")
            nc.scalar.activation(warm2[:], warm[:], EXP, scale=1.0)

            def dmab(q, sb, d, c0, c1):
                q.dma_start(sb[:, c0:c1], d.ap()[:, c0:c1])

            # ---- input DMA: two FIFO rings, ordered by first use ----
            # sync ring: xk sliver halves + wk halves (K proj k-chases),
            #   then xk rest blocks, mpad, xv blocks.
            # scalar ring (must finish issuing before the first exp):
            #   wq halves + xq block0 halves (Q proj c0), xq block1, wv,
            #   xq blocks 2-3.
            kb0 = KBLK[0][2] + KT * (KBLK[0][1] - KBLK[0][0])  # end of blk 0
            qb = [b[2] + KT * (b[1] - b[0]) for b in QBLK]     # block ends
            vb = [b[2] + KT * (b[1] - b[0]) for b in VBLK]
            kb = [b[2] + KT * (b[1] - b[0]) for b in KBLK]
            dmab(QS, xk_all, xk_d, 0, kb0 // 2)
            dmab(QA, wq_sb, wq_d, 0, KT * GD // 2)
            dmab(QS, wk_sb, wk_d, 0, KT * GD // 2)
            dmab(QA, xq_all, xq_d, 0, qb[0] // 2)
            dmab(QS, xk_all, xk_d, kb0 // 2, kb0)
            dmab(QA, wq_sb, wq_d, KT * GD // 2, KT * GD)
            dmab(QS, wk_sb, wk_d, KT * GD // 2, KT * GD)
            dmab(QA, xq_all, xq_d, qb[0] // 2, qb[0])
            if len(KBLK) > 1:
                dmab(QS, xk_all, xk_d, kb0, kb[1])
            dmab(QA, xq_all, xq_d, qb[0], qb[1])
            if len(KBLK) > 2:
                dmab(QS, xk_all, xk_d, kb[1], kb[2])
            QS.dma_start(mpad_sb[:], mpad_d.ap())
            dmab(QA, wv_sb, wv_d, 0, KT * GD)
            dmab(QS, xv_all, xv_d, 0, vb[0])
            dmab(QA, xq_all, xq_d, qb[1], qb[2])
            if len(VBLK) > 1:
                dmab(QS, xv_all, xv_d, vb[0], vb[1])
            dmab(QA, xq_all, xq_d, qb[2], qb[3])
            if use_bias:
                bq_c = small_p.tile([128, 2], F32, tag="biasq")
                QS.dma_start(
                    bq_c[:].rearrange("p (o x) -> p o x", o=2),
                    bq_d.ap().rearrange("(o p) x -> p o x", p=128))
                bk_c = small_p.tile([128, 2], F32, tag="biask")
                QS.dma_start(
                    bk_c[:].rearrange("p (o x) -> p o x", o=2),
                    bk_d.ap().rearrange("(o p) x -> p o x", p=128))
                bv_row = small_p.tile([1, GD], BF16, tag="bvrow")
                QS.dma_start(bv_row[:], bv_d.ap())
                ones_row = small_p.tile([1, 128], BF16, tag="ones")
                nc.vector.memset(ones_row[:], 1.0)

            # ---- projections ----
            qT = {}   # per otile: [128, S] bf16  (o on partitions)
            kTt = {}  # per otile: [128, J] bf16

            def proj_finish(dst, bias_col, ot, ps, o0, n):
                if use_bias:
                    nc.vector.tensor_scalar(
                        dst[:, o0:o0 + n], ps[:, 0:n],
                        bias_col[:, ot:ot + 1], None,
                        mybir.AluOpType.add)
                else:
                    nc.vector.tensor_copy(dst[:, o0:o0 + n], ps[:, 0:n])

            def proj_pass(x_s, w_sb, dst, bias_col, ot, chunk_group):
                """One k-outer accumulation pass over <=2 width-chunks."""
                ps = [pps_p.tile([128, 512], F32, tag="pps",
                                 name=f"pps{ot}{o0}") for (o0, _) in chunk_group]
                for k in range(KT):
                    lw = w_sb[:, k * GD + ot * 128:k * GD + (ot + 1) * 128]
                    for ci, (o0, n) in enumerate(chunk_group):
                        nc.tensor.matmul(ps[ci][:, 0:n], lhsT=lw,
                                         rhs=x_s(k, o0, n),
                                         start=(k == 0), stop=(k == KT - 1))
                for ci, (o0, n) in enumerate(chunk_group):
                    proj_finish(dst, bias_col, ot, ps[ci], o0, n)

            def proj_passes(x_s, w_sb, dst_map, bias_col, width, ot,
                            chunk_list=None):
                """Returns (cost_ns, closure) filler pieces, one per chunk
                pass."""
                if ot in dst_map:
                    dst = dst_map[ot]
                else:
                    dst = qk_p.tile([128, width], BF16,
                                    tag="qt" if width == S else "kt",
                                    name=f"qk{ot}")
                    dst_map[ot] = dst
                if chunk_list is None:
                    chunk_list = _chunks(width, 512)
                return [
                    (int(KT * c[1] * 0.42),
                     lambda cg=cg: proj_pass(x_s, w_sb, dst, bias_col,
                                             ot, cg))
                    for c, cg in [(c, [c]) for c in chunk_list]
                ]

            vext = [None] * NJT

            def v_wave(jts):
                """V projection (flipped orientation) for a couple of j-tiles
                + V_ext assembly."""
                ps = [pps_p.tile([128, GD], F32, tag="pps", name=f"ppsv{jt}")
                      for jt in jts]
                for k in range(KT):
                    for vi, jt in enumerate(jts):
                        nc.tensor.matmul(
                            ps[vi][:, :],
                            lhsT=xv_s(k, jt * 128, 128),
                            rhs=wv_sb[:, k * GD:(k + 1) * GD],
                            start=(k == 0),
                            stop=(k == KT - 1) and not use_bias)
                for vi, jt in enumerate(jts):
                    if use_bias:
                        nc.tensor.matmul(ps[vi][:, :], lhsT=ones_row[:, :],
                                         rhs=bv_row[:, :], start=False,
                                         stop=True)
                    ve = vext_p.tile([128, GH * VW1], BF16, tag="vext",
                                     name=f"vext{jt}")
                    for h in range(GH):
                        nc.vector.tensor_copy(
                            ve[:, h * VW1:h * VW1 + HD],
                            ps[vi][:, h * HD:(h + 1) * HD])
                        nc.vector.tensor_copy(
                            ve[:, h * VW1 + HD:h * VW1 + HD + 1],
                            mpad_sb[:, jt:jt + 1])
                    vext[jt] = ve

            def emit_qkt(ib, hp, fillers, start_jt=3, split_jts=(),
                         mid_fillers=None):
                """Scores + exp for head pair hp of i-block ib. The A/B head
                matmuls go to disjoint PE row halves (explicit tile_position)
                so the pair streams concurrently at ~2x. Emits one filler
                closure after each j-tile (from start_jt on) to keep PE fed
                while ACT chews on the exps."""
                i0 = ib * IBLK
                pt = {}
                for jt in range(NJT):
                    psA = sps_p.tile([128, IBLK], F32, tag="sps",
                                     name=f"sA{ib}{hp}{jt}")
                    psB = sps_p.tile([128, IBLK], F32, tag="sps",
                                     name=f"sB{ib}{hp}{jt}")
                    ptp = pt_p.tile([128, 2 * IBLK], BF16, tag="pt",
                                    name=f"pt{ib}{hp}{jt}")
                    if jt in split_jts:
                        # chunk-paced: exp each 512-wide half as soon as its
                        # scores land (first-exp latency, front only)
                        for (o, n) in _chunks(IBLK, 512):
                            nc.tensor.matmul(
                                psA[:, o:o + n],
                                lhsT=kTt[hp][0:64, jt * 128:(jt + 1) * 128],
                                rhs=qT[hp][0:64, i0 + o:i0 + o + n],
                                start=True, stop=True, tile_position=(0, 0))
                            nc.tensor.matmul(
                                psB[:, o:o + n],
                                lhsT=kTt[hp][64:128, jt * 128:(jt + 1) * 128],
                                rhs=qT[hp][64:128, i0 + o:i0 + o + n],
                                start=True, stop=True, tile_position=(64, 0))
                            nc.scalar.activation(ptp[:, o:o + n],
                                                 psA[:, o:o + n], EXP,
                                                 scale=scale)
                            nc.scalar.activation(
                                ptp[:, IBLK + o:IBLK + o + n],
                                psB[:, o:o + n], EXP, scale=scale)
                            while mid_fillers:
                                mid_fillers.pop(0)[1]()
                        pt[(hp * 2, jt)] = (ptp, 0)
                        pt[(hp * 2 + 1, jt)] = (ptp, IBLK)
                        continue
                    for (o, n) in _chunks(IBLK, 512):
                        nc.tensor.matmul(
                            psA[:, o:o + n],
                            lhsT=kTt[hp][0:64, jt * 128:(jt + 1) * 128],
                            rhs=qT[hp][0:64, i0 + o:i0 + o + n],
                            start=True, stop=True, tile_position=(0, 0))
                        nc.tensor.matmul(
                            psB[:, o:o + n],
                            lhsT=kTt[hp][64:128, jt * 128:(jt + 1) * 128],
                            rhs=qT[hp][64:128, i0 + o:i0 + o + n],
                            start=True, stop=True, tile_position=(64, 0))
                    nc.scalar.activation(ptp[:, 0:IBLK], psA[:], EXP,
                                         scale=scale)
                    nc.scalar.activation(ptp[:, IBLK:2 * IBLK], psB[:], EXP,
                                         scale=scale)
                    pt[(hp * 2, jt)] = (ptp, 0)
                    pt[(hp * 2 + 1, jt)] = (ptp, IBLK)
                    if jt >= start_jt and fillers:
                        slots_left = NJT - jt
                        budget = sum(c for c, _ in fillers) // slots_left
                        spent = 0
                        while fillers and spent < budget:
                            c, fn = fillers.pop(0)
                            fn()
                            spent += c
                while fillers:
                    fillers.pop(0)[1]()
                return pt

            ost_cur = {}
            pv_cur = {}
            oq = [nc.gpsimd, nc.sync]
            oqi = [0]

            def stage_out(osb, h, ib, icl, pv):
                """Copy one 512-wide half of a head's output to SBUF and DMA
                it immediately on an alternating queue."""
                nc.vector.tensor_copy(osb[:, icl * 512:(icl + 1) * 512],
                                      pv[:, :])
                q = oq[oqi[0] % 2]
                oqi[0] += 1
                q.dma_start(
                    out_d.ap()[h * VW1:(h + 1) * VW1,
                               ib * IBLK + icl * 512:ib * IBLK
                               + (icl + 1) * 512],
                    osb[:, icl * 512:(icl + 1) * 512])

            def pv_group(ib, hp, hl, icl, pt, jt_lo, jt_hi, first, last):
                """Part of one PV accumulation group (j-tiles jt_lo:jt_hi);
                on the last part the numerators + denominator row are staged
                per head and DMA'd out unnormalized (host folds the division
                into unsharding)."""
                h = hp * 2 + hl
                if first:
                    pv_cur[(ib, h, icl)] = pps_p.tile(
                        [VW1, 512], F32, tag="pps", name=f"pv{ib}{h}{icl}")
                pv = pv_cur[(ib, h, icl)]
                for jt in range(jt_lo, jt_hi):
                    tp, off = pt[(h, jt)]
                    nc.tensor.matmul(
                        pv[:, :],
                        lhsT=vext[jt][:, h * VW1:(h + 1) * VW1],
                        rhs=tp[:, off + icl * 512:off + (icl + 1) * 512],
                        start=(jt == 0), stop=(jt == NJT - 1))
                if not last:
                    return
                if (ib, h) not in ost_cur:
                    ost_cur[(ib, h)] = ost_p.tile([VW1, IBLK], BF16,
                                                  tag="ost", name=f"o{ib}{h}")
                stage_out(ost_cur[(ib, h)], h, ib, icl, pv)

            # ---- emission schedule ----
            bqc = bq_c if use_bias else None
            bkc = bk_c if use_bias else None
            # K proj passes aligned to the xk DMA blocks.
            kchunks = [(lo, hi - lo) for (lo, hi, _) in KBLK]
            j0 = kchunks[0][1]
            k0_passes = proj_passes(xk_s, wk_sb, kTt, bkc, J, 0, kchunks)
            q0_passes = proj_passes(xq_s, wq_sb, qT, bqc, S, 0,
                                    _chunks(S, 512))
            # Pre-phase PE, interleaved at half-k granularity so the K jt0-1
            # and Q i0:512 passes chase the split weight/input DMAs.
            ps_k = pps_p.tile([128, 512], F32, tag="pps", name="prek")
            ps_q = pps_p.tile([128, 512], F32, tag="pps", name="preq")

            def pre_half(ps, w_sb, x_s, ot, o0, n, k_lo, k_hi):
                for k in range(k_lo, k_hi):
                    lw = w_sb[:, k * GD + ot * 128:k * GD + (ot + 1) * 128]
                    nc.tensor.matmul(ps[:, 0:n], lhsT=lw, rhs=x_s(k, o0, n),
                                     start=(k == 0), stop=(k == KT - 1))

            pre_half(ps_k, wk_sb, xk_s, 0, 0, j0, 0, KT // 2)
            pre_half(ps_q, wq_sb, xq_s, 0, 0, 512, 0, KT // 2)
            pre_half(ps_k, wk_sb, xk_s, 0, 0, j0, KT // 2, KT)
            proj_finish(kTt[0], bkc, 0, ps_k, 0, j0)
            pre_half(ps_q, wq_sb, xq_s, 0, 0, 512, KT // 2, KT)
            proj_finish(qT[0], bqc, 0, ps_q, 0, 512)

            v_fillers = [(int(KT * GD * 0.42),
                          lambda js=js: v_wave(js))
                         for js in [(jt,) for jt in range(NJT)]]
            q1_fillers = proj_passes(xq_s, wq_sb, qT, bqc, S, 1,
                                     _chunks(S, 512))
            k1_fillers = proj_passes(xk_s, wk_sb, kTt, bkc, J, 1, kchunks)

            def pv_fillers(ib, hp, pt):
                out = []
                cuts = [0, NJT // 3, 2 * NJT // 3, NJT]
                for hl in range(2):
                    for icl in range(IBLK // 512):
                        for ci in range(3):
                            lo, hi = cuts[ci], cuts[ci + 1]
                            out.append((int((hi - lo) * 512 * 0.42),
                                        lambda hl=hl, icl=icl, lo=lo, hi=hi,
                                        ci=ci:
                                        pv_group(ib, hp, hl, icl, pt,
                                                 lo, hi, ci == 0, ci == 2)))
                return out

            def pv_tail(ib, hp, pt):
                """Final-phase PV: the two icl-groups of each head accumulate
                in lockstep per j-tile so only the last j-tile's matmuls
                trail the final exps."""
                # all four chains advance per j-tile so only the last
                # exp's head trails; h1's chains borrow the score-psum pool
                # (its buffers idle as the final exps drain them)
                pvs = {}
                for icl in range(IBLK // 512):
                    pvs[(0, icl)] = pps_p.tile([VW1, 512], F32, tag="pps",
                                               name=f"pvt0{icl}")
                for icl in range(IBLK // 512):
                    pvs[(1, icl)] = sps_p.tile([VW1, 512], F32, tag="sps",
                                               name=f"pvt1{icl}")
                for jt in range(NJT):
                    for hl in range(2):
                        h = hp * 2 + hl
                        for icl in range(IBLK // 512):
                            tp, off = pt[(h, jt)]
                            nc.tensor.matmul(
                                pvs[(hl, icl)][:, :],
                                lhsT=vext[jt][:, h * VW1:(h + 1) * VW1],
                                rhs=tp[:, off + icl * 512:
                                       off + (icl + 1) * 512],
                                start=(jt == 0), stop=(jt == NJT - 1))
                for hl in range(2):
                    h = hp * 2 + hl
                    osb = ost_p.tile([VW1, IBLK], BF16, tag="ost",
                                     name=f"ot{h}")
                    for icl in range(IBLK // 512):
                        stage_out(osb, h, ib, icl, pvs[(hl, icl)])

            if pipelined:
                # Front = K0 jt0-1 + Q0 i0:512 (interleaved above); the jt0
                # exps are chunk-paced and Q0 i512:1024 + K0 jt2-5 ride as
                # mid-fillers between the first exp chunks.  Everything else
                # is filler inside a later phase's exp window, ordered by
                # both DMA landing time and first use.  IMPORTANT: a kT/qT
                # pass must be EMITTED before any scores that read it (the
                # PE queue is in-order).
                pt00 = emit_qkt(0, 0,
                                k0_passes[2:] + k1_fillers
                                + q1_fillers[:2] + q0_passes[2:]
                                + v_fillers[:1],
                                start_jt=0, split_jts=(0, 1),
                                mid_fillers=[q0_passes[1], k0_passes[1]])
                g00 = pv_fillers(0, 0, pt00)
                pt10 = emit_qkt(1, 0, v_fillers[1:] + g00, start_jt=0)
                g10 = pv_fillers(1, 0, pt10)
                pt01 = emit_qkt(0, 1, g10 + q1_fillers[2:], start_jt=0)
                g01 = pv_fillers(0, 1, pt01)
                pt11 = emit_qkt(1, 1, g01, start_jt=0)
                pv_tail(1, 1, pt11)
            else:
                for _, p in [q0_passes[1]] + k0_passes[1:] + q0_passes[2:]:
                    p()
                pt00 = emit_qkt(0, 0, [])
                for _, f in v_fillers + q1_fillers + k1_fillers:
                    f()
                for _, f in pv_fillers(0, 0, pt00):
                    f()
                pt10 = emit_qkt(1, 0, [])
                for _, f in pv_fillers(1, 0, pt10):
                    f()
                pt01 = emit_qkt(0, 1, [])
                for _, f in pv_fillers(0, 1, pt01):
                    f()
                pt11 = emit_qkt(1, 1, [])
                pv_tail(1, 1, pt11)

    nc.compile()
    return nc


def _prep_and_run(inputs, trace=False):
    query = np.asarray(inputs["query"], dtype=np.float32)
    key = np.asarray(inputs["key"], dtype=np.float32)
    value = np.asarray(inputs["value"], dtype=np.float32)
    mask = np.asarray(inputs["mask"]).reshape(S)
    w_q = np.asarray(inputs["w_q"], dtype=np.float32)
    b_q = np.asarray(inputs["b_q"], dtype=np.float32)
    w_k = np.asarray(inputs["w_k"], dtype=np.float32)
    b_k = np.asarray(inputs["b_k"], dtype=np.float32)
    w_v = np.asarray(inputs["w_v"], dtype=np.float32)
    b_v = np.asarray(inputs["b_v"], dtype=np.float32)

    use_bias = bool(np.any(b_q) or np.any(b_k) or np.any(b_v))

    # compact key/value over masked-out positions
    idx = np.nonzero(mask != 0)[0]
    J_real = int(len(idx))
    assert J_real > 0, "all positions masked: softmax undefined"
    J = max(512, ((J_real + 127) // 128) * 128)
    key_c = np.zeros((J, B, D), np.float32)
    key_c[:J_real] = key[idx]
    value_c = np.zeros((J, B, D), np.float32)
    value_c[:J_real] = value[idx]

    bf = ml_dtypes.bfloat16

    def wrearr(w):
        # [GD, D] -> w.T [D, GD] -> [128, KT*GD] (k-tiles side by side)
        KT_, GD_ = 8, w.shape[0]
        return np.ascontiguousarray(
            w.T.reshape(KT_, 128, GD_).transpose(1, 0, 2).reshape(
                128, KT_ * GD_)).astype(bf)

    NJT = J // 128
    mflat = np.zeros(J, np.float32)
    mflat[:J_real] = 1  # mpad[p, t] = 1 iff t*128+p < J_real
    mpad = np.ascontiguousarray(mflat.reshape(NJT, 128).T).astype(bf)

    def pack(xT, blks):
        """[D, W] -> [128, KT*W] in the k-interleaved blocked layout the
        kernel's single-tile accessors expect (must match _blocks)."""
        r = xT.reshape(KT, 128, xT.shape[1])
        parts = [r[:, :, lo:hi].transpose(1, 0, 2).reshape(128, -1)
                 for (lo, hi, _) in blks]
        return np.ascontiguousarray(np.concatenate(parts, axis=1)).astype(bf)

    QBLK = _blocks(S, (512, 512, 512, 512))
    KBLK = _blocks(J, (256, 512, 384))
    VBLK = _blocks(J, (640, 512))
    in_maps = []
    for core in range(N_CORES):
        b = core // GROUPS
        g = core % GROUPS
        hs = slice(g * GD, (g + 1) * GD)
        m = {
            "xq": pack(query[:, b, :].T, QBLK),
            "xk": pack(key_c[:, b, :].T, KBLK),
            "xv": pack(value_c[:, b, :].T, VBLK),
            "wq": wrearr(w_q[hs, :]),
            "wk": wrearr(w_k[hs, :]),
            "wv": wrearr(w_v[hs, :]),
            "mpad": mpad,
        }
        if use_bias:
            m["bq"] = np.ascontiguousarray(b_q[hs]).reshape(GD, 1)
            m["bk"] = np.ascontiguousarray(b_k[hs]).reshape(GD, 1)
            m["bv"] = np.ascontiguousarray(b_v[hs]).reshape(1, GD).astype(bf)
        in_maps.append(m)

    ck = (J, J_real, use_bias)
    if ck not in _CACHE:
        _CACHE[ck] = _build(J, J_real, use_bias)
    nc = _CACHE[ck]

    kwargs = {}
    if trace:
        kwargs = dict(trace=True, trace_cores=list(range(N_CORES)))
    res = run_bass_kernel_spmd(nc, in_maps, core_ids=list(range(N_CORES)),
                               **kwargs)

    out = np.empty((S, B, D), np.float32)
    for core in range(N_CORES):
        b = core // GROUPS
        g = core % GROUPS
        r = np.asarray(res.results[core]["out"]).astype(np.float32)
        r = r.reshape(GH, VW1, S)
        out[:, b, g * GD:(g + 1) * GD] = (
            (r[:, :HD, :] / r[:, HD:HD + 1, :])     # softmax denominator
            .reshape(GD, S).T)
    return out, res


def kernel(**inputs):
    out, _ = _prep_and_run(inputs, trace=False)
    return out


def run_traced(**inputs):
    _, res = _prep_and_run(inputs, trace=True)
    return res
